# revision 9
# baseline (speedup 1.0000x reference)
"""Trainium2 Bass kernel for gated multi-head attention (nn_Attention_71751723647784).

Reference computation (B=1, Q=K=2048, CQ=CK=CV=128, H=8, CH=32, HD=256):
    q = (q_x @ Wq)/sqrt(CH); k = kv_x @ Wk; v = kv_x @ Wv           (per-head CH=32)
    a = softmax(q k^T + bias + distance.transpose(0,3,1,2), axis=-1)
    o = (a @ v) * sigmoid(q_x @ Wg + bg);  out = o @ Wo + bo

Sharding: rows of Q across the 8 cores (256 query rows per core). Every input
byte is read exactly once (bias is shared across heads, so head-sharding would
re-read it 8x); no collectives are needed -- each core produces 256 output rows.
"""

import math
import numpy as np

B, Q, KS = 1, 2048, 2048
CQ = 128
H, CH = 8, 32
HD = H * CH  # 256
NCORES = 8
QL = Q // NCORES       # 256 query rows per core
QT = 128               # q-tile (partition dim)
NQT = QL // QT         # 2 q-tiles per core
KC = 512               # k-chunk for score matmuls (one PSUM bank)
NKC = KS // KC         # 4 chunks
SCALE = 1.0 / math.sqrt(CH)

_CACHE = {}


def build_nc():
    from concourse import bacc
    import concourse.tile as tile
    import concourse.bass as bass
    import concourse.mybir as mybir
    from concourse.masks import make_identity

    f32 = mybir.dt.float32
    bf16 = mybir.dt.bfloat16
    AF = mybir.ActivationFunctionType
    ALU = mybir.AluOpType

    nc = bacc.Bacc("TRN2", target_bir_lowering=False, debug=False)

    q_x = nc.dram_tensor("q_x", (QL, CQ), f32, kind="ExternalInput").ap()
    kv_x = nc.dram_tensor("kv_x", (KS, CQ), f32, kind="ExternalInput").ap()
    bias = nc.dram_tensor("bias", (QL, KS), f32, kind="ExternalInput").ap()
    dist = nc.dram_tensor("distance", (QL, KS, H), f32, kind="ExternalInput").ap()
    Wq = nc.dram_tensor("Wq", (CQ, HD), f32, kind="ExternalInput").ap()
    Wk = nc.dram_tensor("Wk", (CQ, HD), f32, kind="ExternalInput").ap()
    Wv = nc.dram_tensor("Wv", (CQ, HD), f32, kind="ExternalInput").ap()
    Wg = nc.dram_tensor("Wg", (CQ, HD), f32, kind="ExternalInput").ap()
    bg = nc.dram_tensor("bg", (HD,), f32, kind="ExternalInput").ap()
    Wo = nc.dram_tensor("Wo", (HD, CQ), f32, kind="ExternalInput").ap()
    bo = nc.dram_tensor("bo", (CQ,), f32, kind="ExternalInput").ap()
    out = nc.dram_tensor("out", (QL, CQ), f32, kind="ExternalOutput").ap()

    with tile.TileContext(nc) as tc:
        with (
            tc.tile_pool(name="const", bufs=1) as constp,
            tc.tile_pool(name="wts", bufs=1) as wtp,
            tc.tile_pool(name="proj", bufs=1) as projp,
            tc.tile_pool(name="dist", bufs=1) as distp,
            tc.tile_pool(name="scores", bufs=3) as scp,
            tc.tile_pool(name="e", bufs=3) as ep,
            tc.tile_pool(name="eT", bufs=6) as etp,
            tc.tile_pool(name="small", bufs=2) as smp,
            tc.tile_pool(name="psA", bufs=4, space="PSUM") as psA,
            tc.tile_pool(name="psO", bufs=2, space="PSUM") as psO,
        ):
            # ---- constants ----
            ident_bf = constp.tile([128, 128], bf16)
            make_identity(nc, ident_bf[:])
            ident_f32 = constp.tile([128, 128], f32)
            make_identity(nc, ident_f32[:])
            ones_bf = constp.tile([1, 128], bf16)
            nc.gpsimd.memset(ones_bf[:], 1.0)

            # ---- weights (cast f32 -> bf16 during DMA, SWDGE) ----
            wq_sb = wtp.tile([128, HD], bf16)
            wk_sb = wtp.tile([128, HD], bf16)
            wv_sb = wtp.tile([128, HD], bf16)
            wg_sb = wtp.tile([128, HD], bf16)
            wo_sb = wtp.tile([128, 2, 128], bf16)
            bo_sb = wtp.tile([1, 128], bf16)
            bg_sb = wtp.tile([128, 2], f32)
            nc.gpsimd.dma_start(wq_sb[:], Wq)
            nc.gpsimd.dma_start(wk_sb[:], Wk)
            nc.gpsimd.dma_start(wv_sb[:], Wv)
            nc.gpsimd.dma_start(wg_sb[:], Wg)
            nc.gpsimd.dma_start(wo_sb[:], Wo.rearrange("(g p) c -> p g c", p=128))
            nc.gpsimd.dma_start(bo_sb[:], bo.rearrange("(a c) -> a c", a=1))
            nc.sync.dma_start(bg_sb[:], bg.rearrange("(g p) -> p g", p=128))

            # ---- activations: load f32->bf16, then transpose on PE ----
            qx_bf = projp.tile([128, NQT, 128], bf16)
            nc.gpsimd.dma_start(qx_bf[:], q_x.rearrange("(a p) c -> p a c", p=128))
            kvx_bf = projp.tile([128, 16, 128], bf16)
            nc.gpsimd.dma_start(kvx_bf[:], kv_x.rearrange("(a p) c -> p a c", p=128))

            qxT = projp.tile([128, QL], bf16)      # [CQ, QL]
            for i in range(NQT):
                ps = psA.tile([128, 128], bf16, tag="psA")
                nc.tensor.transpose(ps[:], qx_bf[:, i, :], ident_bf[:])
                nc.vector.tensor_copy(qxT[:, i * 128:(i + 1) * 128], ps[:])
            kvxT = projp.tile([128, KS], bf16)     # [CQ, K]
            for i in range(16):
                ps = psA.tile([128, 128], bf16, tag="psA")
                nc.tensor.transpose(ps[:], kvx_bf[:, i, :], ident_bf[:])
                nc.vector.tensor_copy(kvxT[:, i * 128:(i + 1) * 128], ps[:])

            # ---- projections ----
            # qT[hd, q] (scaled by 1/sqrt(CH)), kT[hd, k], per hd-half g
            qT = [projp.tile([128, QL], bf16, tag=f"qT{g}", name=f"qT{g}") for g in range(2)]
            kT = [projp.tile([128, KS], bf16, tag=f"kT{g}", name=f"kT{g}") for g in range(2)]
            for g in range(2):
                ps = psA.tile([128, 256], f32, tag="psA")
                nc.tensor.matmul(ps[:], lhsT=wq_sb[:, g * 128:(g + 1) * 128],
                                 rhs=qxT[:], start=True, stop=True)
                nc.scalar.activation(qT[g][:], ps[:], AF.Copy, scale=SCALE)
                for c in range(NKC):
                    ps2 = psA.tile([128, KC], f32, tag="psA")
                    nc.tensor.matmul(ps2[:], lhsT=wk_sb[:, g * 128:(g + 1) * 128],
                                     rhs=kvxT[:, c * KC:(c + 1) * KC],
                                     start=True, stop=True)
                    nc.scalar.copy(kT[g][:, c * KC:(c + 1) * KC], ps2[:])
            # v[k, hd] in 16 k-tiles
            v_sb = projp.tile([128, 16, HD], bf16)
            for kt in range(16):
                ps = psA.tile([128, HD], f32, tag="psA")
                nc.tensor.matmul(ps[:], lhsT=kvxT[:, kt * 128:(kt + 1) * 128],
                                 rhs=wv_sb[:], start=True, stop=True)
                nc.vector.tensor_copy(v_sb[:, kt, :], ps[:])
            # gT[hd, q] = sigmoid(Wg^T qx + bg), per (qt, half)
            gT = [[projp.tile([128, 128], bf16, tag=f"gT{qt}{g}", name=f"gT{qt}{g}") for g in range(2)]
                  for qt in range(NQT)]
            for qt in range(NQT):
                for g in range(2):
                    ps = psA.tile([128, 128], f32, tag="psA")
                    nc.tensor.matmul(ps[:], lhsT=wg_sb[:, g * 128:(g + 1) * 128],
                                     rhs=qxT[:, qt * 128:(qt + 1) * 128],
                                     start=True, stop=True)
                    nc.scalar.activation(gT[qt][g][:], ps[:], AF.Sigmoid,
                                         bias=bg_sb[:, g:g + 1])

            # ---- main attention loop ----
            for qt in range(NQT):
                # distance slice (interleaved (k,h)) + bias, cast to bf16 on load
                dist_sb = distp.tile([128, KS * H], bf16, tag=f"dist{qt}")
                dview = dist.rearrange("(a p) k h -> a p (k h)", p=128)
                for c in range(NKC):
                    nc.gpsimd.dma_start(
                        dist_sb[:, c * KC * H:(c + 1) * KC * H],
                        dview[qt, :, c * KC * H:(c + 1) * KC * H])
                bias_bf = distp.tile([128, KS], bf16, tag=f"bias{qt}")
                nc.gpsimd.dma_start(
                    bias_bf[:], bias.rearrange("(a p) k -> a p k", p=128)[qt])

                denom = smp.tile([128, H], f32, tag="denom")
                recip = smp.tile([128, H], f32, tag="recip")
                dist3 = dist_sb[:].rearrange("p (k h) -> p k h", h=H)

                pso = [None, None]
                for h in range(H):
                    g, hl = h // 4, h % 4
                    score = scp.tile([128, KS], f32, tag="score")
                    for c in range(NKC):
                        ps = psA.tile([128, KC], f32, tag="psA")
                        # bias enters PSUM via identity matmul, then QK accumulates
                        nc.tensor.matmul(ps[:], lhsT=ident_bf[:],
                                         rhs=bias_bf[:, c * KC:(c + 1) * KC],
                                         start=True, stop=False)
                        nc.tensor.matmul(
                            ps[:],
                            lhsT=qT[g][32 * hl:32 * hl + 32, qt * 128:(qt + 1) * 128],
                            rhs=kT[g][32 * hl:32 * hl + 32, c * KC:(c + 1) * KC],
                            start=False, stop=True, tile_position=(32 * hl, 0))
                        # score = psum(qk+bias) + dist  (strided bf16 read)
                        nc.vector.scalar_tensor_tensor(
                            out=score[:, c * KC:(c + 1) * KC], in0=ps[:],
                            scalar=1.0, in1=dist3[:, c * KC:(c + 1) * KC, h],
                            op0=ALU.mult, op1=ALU.add)
                    e_sb = ep.tile([128, KS], bf16, tag="e")
                    nc.scalar.activation(e_sb[:], score[:], AF.Exp,
                                         accum_out=denom[:, h:h + 1])
                    # normalize rows while q is on partitions (DVE 4x bf16),
                    # then transpose e -> eT[kl, kt, q] via xbar DMA
                    nc.vector.reciprocal(recip[:, h:h + 1], denom[:, h:h + 1])
                    e_n = ep.tile([128, KS], bf16, tag="en")
                    nc.vector.tensor_scalar_mul(e_n[:], e_sb[:], recip[:, h:h + 1])
                    et = etp.tile([128, 16, 128], bf16, tag="eT")
                    nc.sync.dma_start_transpose(et[:], e_n[:])
                    # AV: 4 heads share one PSUM bank (col-tiled partition
                    # ranges); heads run sequentially within the bank so the
                    # start=True bank-clear never interleaves accumulations.
                    if hl == 0:
                        pso[g] = psO.tile([128, 128], f32, tag="psO",
                                          name=f"pso{qt}{g}")
                    for kt in range(16):
                        nc.tensor.matmul(
                            pso[g][32 * hl:32 * hl + 32, :],
                            lhsT=v_sb[:, kt, 32 * h:32 * h + 32],
                            rhs=et[:, kt, :],
                            start=(kt == 0), stop=(kt == 15),
                            tile_position=(0, 32 * hl))

                gos = []
                for hg in range(2):
                    go = smp.tile([128, 128], bf16, tag="go")
                    nc.vector.tensor_mul(go[:], pso[hg][:], gT[qt][hg][:])
                    gos.append(go)

                # final projection: out[q, co] = sum_hd go[hd, q] * Wo[hd, co] + bo
                psout = psA.tile([128, 128], f32, tag="psA")
                nc.tensor.matmul(psout[:], lhsT=gos[0][:], rhs=wo_sb[:, 0, :],
                                 start=True, stop=False)
                nc.tensor.matmul(psout[:], lhsT=gos[1][:], rhs=wo_sb[:, 1, :],
                                 start=False, stop=False)
                nc.tensor.matmul(psout[:], lhsT=ones_bf[:], rhs=bo_sb[:],
                                 start=False, stop=True)
                out_sb = smp.tile([128, 128], f32, tag="out")
                nc.vector.tensor_copy(out_sb[:], psout[:])
                nc.sync.dma_start(
                    out.rearrange("(a p) c -> a p c", p=128)[qt], out_sb[:])

    nc.compile()
    return nc


def _get_nc():
    if "nc" not in _CACHE:
        _CACHE["nc"] = build_nc()
    return _CACHE["nc"]


def make_in_maps(q_x, kv_x, bias, distance, Wq, Wk, Wv, Wg, bg, Wo, bo):
    com = {
        "kv_x": np.ascontiguousarray(kv_x[0]),
        "Wq": np.ascontiguousarray(Wq), "Wk": np.ascontiguousarray(Wk),
        "Wv": np.ascontiguousarray(Wv), "Wg": np.ascontiguousarray(Wg),
        "bg": np.ascontiguousarray(bg), "Wo": np.ascontiguousarray(Wo),
        "bo": np.ascontiguousarray(bo),
    }
    maps = []
    for i in range(NCORES):
        s = slice(i * QL, (i + 1) * QL)
        m = dict(com)
        m["q_x"] = np.ascontiguousarray(q_x[0, s])
        m["bias"] = np.ascontiguousarray(bias[0, 0, s])
        m["distance"] = np.ascontiguousarray(distance[0, s])
        maps.append(m)
    return maps


def kernel(q_x, kv_x, bias, distance, Wq, Wk, Wv, Wg, bg, Wo, bo, trace=False):
    from concourse.bass_utils import run_bass_kernel_spmd

    nc = _get_nc()
    in_maps = make_in_maps(np.asarray(q_x, np.float32), np.asarray(kv_x, np.float32),
                           np.asarray(bias, np.float32),
                           np.asarray(distance, np.float32),
                           np.asarray(Wq, np.float32), np.asarray(Wk, np.float32),
                           np.asarray(Wv, np.float32), np.asarray(Wg, np.float32),
                           np.asarray(bg, np.float32), np.asarray(Wo, np.float32),
                           np.asarray(bo, np.float32))
    res = run_bass_kernel_spmd(nc, in_maps, core_ids=list(range(NCORES)),
                               trace=trace)
    _CACHE["last_result"] = res
    out = np.concatenate([res.results[i]["out"] for i in range(NCORES)], axis=0)
    return out.reshape(B, Q, CQ).astype(np.float32)
